# revision 14
# baseline (speedup 1.0000x reference)
"""Chamfer loss kernel for Trainium2 (8 NeuronCores, SPMD).

Strategy
--------
s[n, m] = 2<x_n, y_m> - ||x_n||^2 - ||y_m||^2  (= -squared distance, <= 0)
dist1[n] = -max_m s[n, m]; dist2[m] = -max_n s[n, m].

Sharding: 8 cores = 4 batches x 2 pred-halves. Core c handles batch c//2,
pred rows [ (c%2)*4096, +4096 ), all 8192 gt rows.

Precision: the K=3 contraction is lifted to a K=16 fp16 matmul via hi/lo
fp16 splitting of coords and norms (all products exact in the fp32 PSUM
accumulator; total error ~1e-5 absolute on s, comparable to an fp32 matmul).

Per core: PE computes s in (128n x 512m) PSUM tiles; ACT downcasts tiles to
an fp16 SBUF "sheet" (128 x 8192) per n-tile; DVE does an elementwise
running max across n-tiles (dist2) and a pairwise-halving max tree along m
(dist1); PE transposes the running-max sheet so DVE can reduce over n for
dist2. Host combines the tiny per-core partials.
"""

import sys

for _p in ("/opt/trn_rl_repo", "/root/.axon_site/_ro/trn_rl_repo"):
    if _p not in sys.path:
        sys.path.insert(0, _p)

import numpy as np

import concourse.bass as bass
import concourse.tile as tile
from concourse import mybir
from concourse.masks import make_identity
from concourse.vector_clock import ScopedClock, VectorClock

FP16 = mybir.dt.float16
FP32 = mybir.dt.float32
NEG_BIG = -60000.0  # fp16-representable, below any s value

# Full-problem geometry
B, N, M = 4, 8192, 8192
N_CORES = 8
N_SHARD = N // 2  # pred rows per core
NT_FULL = N_SHARD // 128  # 32 n-tiles per core
MJ_FULL = M // 512  # 16 m-tiles


def _patched_drain_and_barrier(self, tick_clock, wait_clock):
    # The pinned walrus rejects >N sync waits on a Drain (TPB_CTRL). Put the
    # waits on single-wait nops first, then emit a wait-free drain.
    gc = tick_clock.global_clock
    n = len(gc)
    for s in range(n):
        part = VectorClock([gc[i] if i == s else 0 for i in range(n)])
        if not any(part):
            continue
        nop = self.nc.sync.nop(nofuse=True)
        wait_clock.add_sem_waits(nop.ins, ScopedClock({None: part}))
    drain_inst = self.nc.sync.drain()
    wait_clock.add_sem_waits(
        drain_inst.ins, ScopedClock({None: gc}), ScopedClock({None: gc})
    )
    self.nc.all_engine_barrier()
    popped = self.nc._tile_sem_poison_stack.pop()
    assert popped is self._sem_poison
    self.nc.clear_and_free_semaphores(list(self.sems.allocated().values()))
    self.nc.all_engine_barrier()


tile.TileContext._drain_and_barrier = _patched_drain_and_barrier

_HOIST_ID = [0]


def _hoist_extra_waits(nc, max_waits=1):
    """Walrus in this toolchain rejects instructions with more than one sync
    wait. Move all but one wait of each instruction onto same-engine NoOps
    inserted just before it (engine program order preserves semantics)."""
    for fn in nc.m.functions:
        for blk in fn.blocks:
            insts = blk.instructions
            if not any(
                i.sync_info and len(i.sync_info.on_wait) > max_waits for i in insts
            ):
                continue
            out = []
            for inst in insts:
                si = inst.sync_info
                if si is not None and len(si.on_wait) > max_waits:
                    waits = list(si.on_wait)
                    extra, keep = waits[:-max_waits], waits[-max_waits:]
                    for w in extra:
                        nop = mybir.InstNoOp(
                            name=f"hoistw_{_HOIST_ID[0]}", ins=[], outs=[]
                        )
                        _HOIST_ID[0] += 1
                        nop.engine = inst.engine
                        nop.sync_info = mybir.SyncInfo(on_wait=[w], on_update=[])
                        out.append(nop)
                    inst.sync_info = mybir.SyncInfo(
                        on_wait=keep, on_update=list(si.on_update)
                    )
                out.append(inst)
            blk.instructions = out


def build_nc(nt: int = NT_FULL, mj: int = MJ_FULL, num_devices: int = N_CORES,
             reps: int = 1):
    """Build the per-core Bass program.

    Inputs:  lhsT (16, nt*128) fp16, rhs (16, mj*512) fp16
    Outputs: d1 (128, nt) fp32   [d1[p, t] = max_m s for n-local = t*128+p]
             d2 (128, mj*4) fp32 [d2[p, g] = max_n s for m = g*128+p]

    reps > 1 repeats the whole computation in one NEFF (for timing deltas).
    """
    n_cols = nt * 128
    m_cols = mj * 512
    n_groups = m_cols // 128

    nc = bass.Bass("TRN2", target_bir_lowering=False, debug=False,
                   num_devices=num_devices)
    lhsT = nc.dram_tensor("lhsT", [16, n_cols], FP16, kind="ExternalInput").ap()
    rhs = nc.dram_tensor("rhs", [16, m_cols], FP16, kind="ExternalInput").ap()
    d1 = nc.dram_tensor("d1", [128, nt], FP32, kind="ExternalOutput").ap()
    d2 = nc.dram_tensor("d2", [128, n_groups], FP32, kind="ExternalOutput").ap()

    from contextlib import ExitStack

    with tile.TileContext(nc) as tc, ExitStack() as ctx:
        consts = ctx.enter_context(tc.tile_pool(name="consts", bufs=1))
        sheets = ctx.enter_context(tc.tile_pool(name="sheets", bufs=3))
        scr4k = ctx.enter_context(tc.tile_pool(name="scr4k", bufs=1))
        scr2k = ctx.enter_context(tc.tile_pool(name="scr2k", bufs=1))
        scr1k = ctx.enter_context(tc.tile_pool(name="scr1k", bufs=2))
        scr512 = ctx.enter_context(tc.tile_pool(name="scr512", bufs=2))
        psmm = ctx.enter_context(tc.tile_pool(name="psmm", bufs=3, space="PSUM"))
        pstr = ctx.enter_context(tc.tile_pool(name="pstr", bufs=2, space="PSUM"))

        lhsT_sb = consts.tile([16, n_cols], FP16)
        rhs_sb = consts.tile([16, m_cols], FP16)
        nc.sync.dma_start(out=lhsT_sb[:], in_=lhsT[:])
        nc.sync.dma_start(out=rhs_sb[:], in_=rhs[:])

        ident = consts.tile([128, 128], FP16)
        make_identity(nc, ident[:])

        half = m_cols // 2
        for _rep in range(reps):
            run2 = consts.tile([128, m_cols], FP16, tag="run2")
            nc.vector.memset(run2[:], NEG_BIG)
            d1cols = consts.tile([128, nt], FP32, tag="d1cols")
            d2cols = consts.tile([128, n_groups], FP32, tag="d2cols")
            _build_body(nc, tc, consts, sheets, scr4k, scr2k, scr1k, scr512,
                        psmm, pstr, lhsT_sb, rhs_sb, ident, run2,
                        d1cols, d2cols, nt, mj, half, n_groups, d1, d2)
    _hoist_extra_waits(nc)
    return nc


def _build_body(nc, tc, consts, sheets, scr4k, scr2k, scr1k, scr512, psmm,
                pstr, lhsT_sb, rhs_sb, ident, run2, d1cols, d2cols, nt, mj,
                half, n_groups, d1, d2):
    m_cols = mj * 512
    assert nt % 2 == 0 and mj % 2 == 0
    for tp in range(nt // 2):  # pair of n-tiles per iteration
        pair = sheets.tile([128, 2, m_cols], FP16)
        for q in range(2):
            t = 2 * tp + q
            for j2 in range(mj // 2):  # 1024-wide PSUM tiles (2 banks)
                ps = psmm.tile([128, 1024], FP32)
                for h in range(2):
                    nc.tensor.matmul(
                        ps[:, h * 512:(h + 1) * 512],
                        lhsT_sb[:, t * 128:(t + 1) * 128],
                        rhs_sb[:, (2 * j2 + h) * 512:(2 * j2 + h + 1) * 512],
                        start=True,
                        stop=True,
                    )
                # ACT: PSUM fp32 -> SBUF fp16, 1024 wide
                nc.scalar.copy(
                    pair[:, q, j2 * 1024:(j2 + 1) * 1024], ps[:]
                )
        # dist2: running max across n-tiles (DVE fp16 2x), one op per sheet
        nc.vector.tensor_max(run2[:], run2[:], pair[:, 0, :])
        nc.vector.tensor_max(run2[:], run2[:], pair[:, 1, :])
        # dist1: pairwise-halving max tree along m for BOTH sheets at once
        a = scr4k.tile([128, 2, half], FP16)
        nc.vector.tensor_max(a[:], pair[:, :, :half], pair[:, :, half:])
        cur = a
        size = half
        scrs = {2048: scr2k, 1024: scr1k, 512: scr512}
        while size > 512:
            size //= 2
            nxt = scrs[size].tile([128, 2, size], FP16)
            nc.vector.tensor_max(nxt[:], cur[:, :, :size], cur[:, :, size:])
            cur = nxt
        nc.vector.tensor_reduce(
            d1cols[:, 2 * tp:2 * tp + 2], cur[:], axis=mybir.AxisListType.X,
            op=mybir.AluOpType.max,
        )
    # dist2 finale: transpose 128-wide m-groups (4 per PSUM tile), then one
    # batched reduce over the n-partition axis per 4 groups
    for g4 in range(n_groups // 4):
        pt = pstr.tile([128, 4, 128], FP16)
        for h in range(4):
            g = 4 * g4 + h
            nc.tensor.transpose(
                pt[:, h, :], run2[:, g * 128:(g + 1) * 128], ident[:]
            )
        nc.vector.tensor_reduce(
            d2cols[:, 4 * g4:4 * g4 + 4], pt[:], axis=mybir.AxisListType.X,
            op=mybir.AluOpType.max,
        )
    nc.sync.dma_start(out=d1[:], in_=d1cols[:])
    nc.sync.dma_start(out=d2[:], in_=d2cols[:])


def _split16(x64):
    """fp64 array -> (hi, lo) fp16 pair with hi+lo ~ x (22-bit capture)."""
    hi = x64.astype(np.float16)
    lo = (x64 - hi.astype(np.float64)).astype(np.float16)
    return hi, lo


def build_lhsT_rhs(x, y):
    """fp16 hi/lo-split matmul operands for point sets x (n,3), y (m,3)."""
    x = np.asarray(x, np.float64)
    y = np.asarray(y, np.float64)
    xh, xl = _split16(x)
    yh, yl = _split16(y)
    nxh, nxl = _split16((x * x).sum(-1))
    nyh, nyl = _split16((y * y).sum(-1))

    lhsT = np.empty((16, x.shape[0]), np.float16)
    rhs = np.empty((16, y.shape[0]), np.float16)
    for cdim in range(3):
        lhsT[0 + cdim] = 2.0 * xh[:, cdim]
        lhsT[3 + cdim] = 2.0 * xh[:, cdim]
        lhsT[6 + cdim] = 2.0 * xl[:, cdim]
        lhsT[9 + cdim] = 2.0 * xl[:, cdim]
        rhs[0 + cdim] = yh[:, cdim]
        rhs[3 + cdim] = yl[:, cdim]
        rhs[6 + cdim] = yh[:, cdim]
        rhs[9 + cdim] = yl[:, cdim]
    lhsT[12] = -nxh
    lhsT[13] = -nxl
    lhsT[14] = 1.0
    lhsT[15] = 1.0
    rhs[12] = 1.0
    rhs[13] = 1.0
    rhs[14] = -nyh
    rhs[15] = -nyl
    return lhsT, rhs


def make_core_inputs(pred, gt):
    """Per-core {lhsT, rhs} fp16 input maps for the full problem."""
    pred = np.asarray(pred, dtype=np.float32)
    gt = np.asarray(gt, dtype=np.float32)
    in_maps = []
    for c in range(N_CORES):
        b, halfi = divmod(c, 2)
        x = pred[b, halfi * N_SHARD:(halfi + 1) * N_SHARD]
        y = gt[b]
        lhsT, rhs = build_lhsT_rhs(x, y)
        in_maps.append({"lhsT": lhsT, "rhs": rhs})
    return in_maps


def combine_outputs(results):
    """Host-side combine of per-core partials -> scalar loss (fp32)."""
    loss = 0.0
    for b in range(B):
        r0, r1 = results[2 * b], results[2 * b + 1]
        # dist1: each core covers its own n rows fully
        s1 = np.concatenate(
            [np.asarray(r0["d1"], np.float64).T.ravel(),
             np.asarray(r1["d1"], np.float64).T.ravel()]
        )  # (N,) ; [t*128+p] ordering via transpose
        pred2gt = (-s1).mean()
        # dist2: max over the two pred halves, then mean over m
        s2 = np.maximum(np.asarray(r0["d2"], np.float64),
                        np.asarray(r1["d2"], np.float64))
        gt2pred = (-s2.T.ravel()).mean()  # m = g*128+p -> transpose
        loss += pred2gt + gt2pred
    return np.array(loss / B, dtype=np.float32)


_NC_CACHE = {}


def kernel(pred, gt):
    from concourse.bass_utils import run_bass_kernel_spmd

    if "nc" not in _NC_CACHE:
        _NC_CACHE["nc"] = build_nc()
    nc = _NC_CACHE["nc"]
    in_maps = make_core_inputs(pred, gt)
    res = run_bass_kernel_spmd(nc, in_maps, list(range(N_CORES)))
    return combine_outputs(res.results)
